# revision 1
# baseline (speedup 1.0000x reference)
"""Trainium2 Bass kernel for CrossModalMultiHeadAttentionK.

Computation (see reference): per-channel 7x7 local attention on a 40x40 grid,
B=2, C=256, with 1x1 convs (q/k/v/out/fuse) and sinusoidal positional
encodings. Sharding: 8 cores = (batch b in {0,1}) x (row-quarter q in {0..3},
10 output rows each). Each core holds all 256 channels in SBUF layout
[128 partitions, 2 channel-slots, spatial] so elementwise attention ops run
with free-dim 800 and no cross-core collectives are needed.

Engine plan per core:
 - PE (fp32): q/k/v 1x1 convs (pe-const folded in as extra accumulation
   matmuls), vo conv, fuse conv; plus fp16 identity-matmuls accumulating
   softmax numerator/denominator over the 49 window offsets into PSUM.
 - DVE (fp16 2x mode): s_j = q*k_j and p_j = e_j*v_j tensor_tensor muls.
   k/v have +1-element-shifted fp16 copies so odd window offsets stay
   4B-aligned (2x_1p requirement).
 - ACT: exp (table-based), PSUM evictions with per-channel bias.
"""

import math
import numpy as np

# ---- problem constants (hardcoded per harness contract) ----
B, C, H, W = 2, 256, 40, 40
KS, PAD = 7, 3
HEAD_DIM = 32
SCALING = HEAD_DIM ** -0.5
TEMPERATURE, PESCALE, EPS = 10000.0, 2.0 * math.pi, 1e-6
NQ = 4                 # row-quarters
RQ = H // NQ           # 10 output rows per core
NPOS = RQ * W          # 400 output positions per slot
KROWS = RQ + KS - 1    # 16 padded rows needed
KW = W + 2 * PAD       # 46 padded cols
KFREE = KROWS * KW     # 736
NF = 800               # 2 slots * NPOS, elementwise free dim
NJ = KS * KS           # 49 window offsets

_CACHE = {}


def _sine_pe(mask):
    """numpy port of reference.sine_pe; mask (b,h,w) bool."""
    nm = (~mask).astype(np.float32)
    y = np.cumsum(nm, axis=1, dtype=np.float32)
    x = np.cumsum(nm, axis=2, dtype=np.float32)
    y = y / (y[:, -1:, :] + EPS) * PESCALE
    x = x / (x[:, :, -1:] + EPS) * PESCALE
    nf = C // 2
    i = np.arange(nf, dtype=np.float32)
    dim_t = (TEMPERATURE ** (2.0 * np.floor(i / 2.0) / nf)).astype(np.float32)
    px = (x[..., None] / dim_t).astype(np.float32)
    py = (y[..., None] / dim_t).astype(np.float32)

    def interleave(p):
        s = np.stack([np.sin(p[..., 0::2]), np.cos(p[..., 1::2])], axis=4)
        return s.reshape(p.shape[0], p.shape[1], p.shape[2], -1)

    pos = np.concatenate([interleave(py), interleave(px)], axis=3)
    return pos.transpose(0, 3, 1, 2).astype(np.float32)  # (b, C, h, w)


def _pe_constants():
    if "pe" in _CACHE:
        return _CACHE["pe"]
    mask_q = np.zeros((1, H, W), dtype=bool)
    pe_q = _sine_pe(mask_q)[0]  # (C, H, W)
    Hp, Wp = H + 2 * PAD, W + 2 * PAD
    mask_k = np.zeros((1, Hp, Wp), dtype=bool)
    mask_k[:, :PAD, :] = True
    mask_k[:, :, :PAD] = True
    mask_k[:, Hp - PAD:, :] = True
    mask_k[:, :, Wp - PAD:] = True
    pe_k = _sine_pe(mask_k)[0]  # (C, Hp, Wp)
    _CACHE["pe"] = (pe_q, pe_k)
    return pe_q, pe_k


def _build_module():
    """Build (once) the per-core Bacc module. Same NEFF on all 8 cores."""
    if "nc" in _CACHE:
        return _CACHE["nc"]
    import concourse.bacc as bacc
    import concourse.tile as tile
    import concourse.mybir as mybir

    f32 = mybir.dt.float32
    f16 = mybir.dt.float16
    AF = mybir.ActivationFunctionType

    nc = bacc.Bacc("TRN2", target_bir_lowering=False, debug=False,
                   enable_asserts=True, num_devices=8)

    din = {}
    for name, shape, dt in [
        ("query", [128, 2, NPOS], f32),
        ("keypad", [128, 2, KFREE], f32),
        ("peq", [128, 2, NPOS], f16),
        ("pek", [128, 2, KFREE], f16),
        ("cf", [128, 2, NPOS], f32),
        ("wq", [2, 128, 256], f16),
        ("wk", [2, 128, 256], f16),
        ("wv", [2, 128, 256], f16),
        ("wo", [2, 128, 256], f32),
        ("wf", [4, 128, 256], f32),
        ("bq", [128, 2], f32),
        ("bk", [128, 2], f32),
        ("bv", [128, 2], f32),
        ("bo", [128, 2], f32),
        ("ident", [128, 128], f16),
    ]:
        din[name] = nc.dram_tensor(name, shape, dt, kind="ExternalInput").ap()
    d_out = nc.dram_tensor("out_part", [128, 2, NPOS], f32, kind="ExternalOutput").ap()
    d_vo = nc.dram_tensor("vo_part", [128, 2, NPOS], f32, kind="ExternalOutput").ap()

    with tile.TileContext(nc) as tc:
        with tc.tile_pool(name="consts", bufs=1) as cp, \
             tc.tile_pool(name="work", bufs=1) as wp, \
             tc.tile_pool(name="sje", bufs=5) as sp, \
             tc.tile_pool(name="psacc", bufs=1, space="PSUM") as pa, \
             tc.tile_pool(name="psconv", bufs=2, space="PSUM") as pc:

            # ---- load inputs ----
            sb = {}
            # spread big input DMAs over several DGE queues so they run in
            # parallel instead of serializing on the Sync queue
            dma_engs = [nc.sync, nc.gpsimd, nc.scalar]
            _di = [0]

            def dma_in(out, in_):
                dma_engs[_di[0] % len(dma_engs)].dma_start(out=out, in_=in_)
                _di[0] += 1

            for name, shape, dt in [
                ("query", [128, 2, NPOS], f32),
                ("keypad", [128, 2, KFREE], f32),
                ("peq", [128, 2, NPOS], f16),
                ("pek", [128, 2, KFREE], f16),
                ("cf", [128, 2, NPOS], f32),
                ("bq", [128, 2], f32),
                ("bk", [128, 2], f32),
                ("bv", [128, 2], f32),
                ("bo", [128, 2], f32),
                ("ident", [128, 128], f16),
            ]:
                t = cp.tile(shape, dt, tag=name)
                dma_in(t[:], din[name][:])
                sb[name] = t
            for name, nk, wdt in [("wq", 2, f16), ("wk", 2, f16), ("wv", 2, f16),
                                  ("wo", 2, f32), ("wf", 4, f32)]:
                tiles = []
                for k in range(nk):
                    t = cp.tile([128, 256], wdt, tag=f"{name}{k}")
                    dma_in(t[:], din[name][k])
                    tiles.append(t)
                sb[name] = tiles

            # fp16 casts of conv moving operands (DVE is idle in preamble)
            query16 = wp.tile([128, 2, NPOS], f16, tag="query16")
            nc.vector.tensor_copy(query16[:], sb["query"][:])
            keypad16 = wp.tile([128, 2, KFREE], f16, tag="keypad16")
            nc.vector.tensor_copy(keypad16[:], sb["keypad"][:])

            # ---- q/k/v convs (pe folded in as extra matmuls) ----
            q_b = wp.tile([128, NF], f16, tag="q_b")
            k_b = wp.tile([128, 2 * KFREE], f16, tag="k_b")
            k_b1 = wp.tile([128, 2 * KFREE], f16, tag="k_b1")
            v_b = wp.tile([128, 2 * KFREE], f16, tag="v_b")
            v_b1 = wp.tile([128, 2 * KFREE], f16, tag="v_b1")

            # q conv: out fp16, scaled weights/bias, pe folded
            for o in range(2):
                ps = pc.tile([128, NPOS], f32, tag="convps")
                for k in range(2):
                    nc.tensor.matmul(ps[:], sb["wq"][k][:, o * 128:(o + 1) * 128],
                                     query16[:, k, :], start=(k == 0), stop=False)
                for k in range(2):
                    nc.tensor.matmul(ps[:], sb["wq"][k][:, o * 128:(o + 1) * 128],
                                     sb["peq"][:, k, :], start=False, stop=(k == 1))
                nc.scalar.activation(out=q_b[:, o * NPOS:(o + 1) * NPOS], in_=ps[:],
                                     func=AF.Identity, bias=sb["bq"][:, o:o + 1])

            # k conv (with pe) and v conv (no pe): 736 free -> 2 chunks of 368
            for name, wname, bias, dest, dest1, with_pe in [
                ("k", "wk", "bk", k_b, k_b1, True),
                ("v", "wv", "bv", v_b, v_b1, False),
            ]:
                for o in range(2):
                    ps = pc.tile([128, KFREE], f32, tag="convps")
                    # psum chunks must not straddle the 2KB bank boundary
                    for sl in (slice(0, 512), slice(512, KFREE)):
                        nmm = 4 if with_pe else 2
                        i = 0
                        for k in range(2):
                            nc.tensor.matmul(ps[:, sl],
                                             sb[wname][k][:, o * 128:(o + 1) * 128],
                                             keypad16[:, k, sl],
                                             start=(i == 0), stop=(i == nmm - 1))
                            i += 1
                        if with_pe:
                            for k in range(2):
                                nc.tensor.matmul(ps[:, sl],
                                                 sb[wname][k][:, o * 128:(o + 1) * 128],
                                                 sb["pek"][:, k, sl],
                                                 start=False, stop=(i == nmm - 1))
                                i += 1
                    nc.scalar.activation(out=dest[:, o * KFREE:(o + 1) * KFREE],
                                         in_=ps[:], func=AF.Identity,
                                         bias=sb[bias][:, o:o + 1])
                # shifted-by-one fp16 copy for odd window offsets (DVE)
                nc.vector.tensor_copy(dest1[:, 0:2 * KFREE - 1], dest[:, 1:2 * KFREE])

            # ---- attention j-loop ----
            # one PSUM tile per (num/den, half) so each matmul output sits in
            # a single bank
            num_ps = [pa.tile([128, NPOS], f32, tag=f"num{h}", name=f"num{h}")
                      for h in range(2)]
            den_ps = [pa.tile([128, NPOS], f32, tag=f"den{h}", name=f"den{h}")
                      for h in range(2)]
            q4 = q_b[:].rearrange("p (a r c) -> p a r c", a=2, r=RQ)
            k4 = k_b[:].rearrange("p (a r c) -> p a r c", a=2, r=KROWS)
            k41 = k_b1[:].rearrange("p (a r c) -> p a r c", a=2, r=KROWS)
            v4 = v_b[:].rearrange("p (a r c) -> p a r c", a=2, r=KROWS)
            v41 = v_b1[:].rearrange("p (a r c) -> p a r c", a=2, r=KROWS)

            for j in range(NJ):
                di, dj = j // KS, j % KS
                if dj % 2 == 0:
                    kv, vv, c0 = k4, v4, dj
                else:
                    kv, vv, c0 = k41, v41, dj - 1
                s_t = sp.tile([128, NF], f16, tag="s")
                s4 = s_t[:].rearrange("p (a r c) -> p a r c", a=2, r=RQ)
                # route a fraction of the qk muls to the otherwise-idle GPSIMD
                s_eng = nc.vector
                s_eng.tensor_mul(s4, q4, kv[:, :, di:di + RQ, c0:c0 + W])
                e_t = sp.tile([128, NF], f16, tag="e")
                nc.scalar.activation(out=e_t[:], in_=s_t[:], func=AF.Exp)
                p_t = sp.tile([128, NF], f16, tag="pp")
                p4 = p_t[:].rearrange("p (a r c) -> p a r c", a=2, r=RQ)
                nc.vector.tensor_mul(p4, e_t[:].rearrange("p (a r c) -> p a r c", a=2, r=RQ),
                                     vv[:, :, di:di + RQ, c0:c0 + W])
                for hh in range(2):
                    sl = slice(hh * NPOS, (hh + 1) * NPOS)
                    nc.tensor.matmul(num_ps[hh][:], sb["ident"][:], p_t[:, sl],
                                     start=(j == 0), stop=(j == NJ - 1))
                    nc.tensor.matmul(den_ps[hh][:], sb["ident"][:], e_t[:, sl],
                                     start=(j == 0), stop=(j == NJ - 1))

            # ---- normalize + vo conv + fuse conv, pipelined by spatial half ----
            HC = NPOS // 2  # 200-position chunks
            r_t = wp.tile([128, NF], f32, tag="r")
            att = wp.tile([128, NF], f32, tag="att")
            vo_sb = wp.tile([128, NF], f32, tag="vo")
            out_sb = wp.tile([128, NF], f32, tag="out")
            for cch in range(2):
                cs = slice(cch * HC, (cch + 1) * HC)
                for hh in range(2):
                    sl = slice(hh * NPOS + cch * HC, hh * NPOS + (cch + 1) * HC)
                    nc.vector.reciprocal(r_t[:, sl], den_ps[hh][:, cs])
                    nc.vector.tensor_mul(att[:, sl], num_ps[hh][:, cs], r_t[:, sl])
                for o in range(2):
                    ps = pc.tile([128, HC], f32, tag="convps", name="tailps")
                    for k in range(2):
                        nc.tensor.matmul(ps[:], sb["wo"][k][:, o * 128:(o + 1) * 128],
                                         att[:, k * NPOS + cch * HC:
                                             k * NPOS + (cch + 1) * HC],
                                         start=(k == 0), stop=(k == 1))
                    nc.scalar.activation(
                        out=vo_sb[:, o * NPOS + cch * HC:o * NPOS + (cch + 1) * HC],
                        in_=ps[:], func=AF.Identity, bias=sb["bo"][:, o:o + 1])
                for o in range(2):
                    ps = pc.tile([128, HC], f32, tag="convps", name="tailps")
                    i = 0
                    for k in range(2):
                        nc.tensor.matmul(ps[:], sb["wf"][k][:, o * 128:(o + 1) * 128],
                                         sb["query"][:, k, cs],
                                         start=(i == 0), stop=False)
                        i += 1
                    for k in range(2):
                        nc.tensor.matmul(ps[:], sb["wf"][2 + k][:, o * 128:(o + 1) * 128],
                                         vo_sb[:, k * NPOS + cch * HC:
                                               k * NPOS + (cch + 1) * HC],
                                         start=False, stop=(i == 3))
                        i += 1
                    # fuse pe contribution folded in host-side (cf)
                    nc.vector.tensor_add(
                        out_sb[:, o * NPOS + cch * HC:o * NPOS + (cch + 1) * HC],
                        ps[:], sb["cf"][:, o, cs])
                nc.sync.dma_start(out=d_vo[:, :, cs], in_=vo_sb[:].rearrange(
                    "p (a n) -> p a n", a=2)[:, :, cs])
                nc.sync.dma_start(out=d_out[:, :, cs], in_=out_sb[:].rearrange(
                    "p (a n) -> p a n", a=2)[:, :, cs])

    nc.compile()
    _CACHE["nc"] = nc
    return nc


def _in_maps(key, query, Wq, bq, Wk, bk, Wv, bv, Wo, bo, Wf):
    pe_q, pe_k = _pe_constants()
    keypad_full = np.pad(key, ((0, 0), (0, 0), (PAD, PAD), (PAD, PAD)))
    wqT = np.ascontiguousarray((Wq.T * SCALING).reshape(2, 128, 256)).astype(np.float16)
    wkT = np.ascontiguousarray(Wk.T.reshape(2, 128, 256)).astype(np.float16)
    wvT = np.ascontiguousarray(Wv.T.reshape(2, 128, 256)).astype(np.float16)
    woT = np.ascontiguousarray(Wo.T.reshape(2, 128, 256)).astype(np.float32)
    wfT = np.ascontiguousarray(Wf.T.reshape(4, 128, 256)).astype(np.float32)
    # fuse-conv pe contribution, folded host-side: Cf = Wf[:, :C] @ pe_q
    cf_full = np.einsum("oc,chw->ohw", Wf[:, :C].astype(np.float32),
                        pe_q).astype(np.float32)  # (C, H, W)
    bq_s = np.ascontiguousarray((bq * SCALING).reshape(2, 128).T).astype(np.float32)
    bk_s = np.ascontiguousarray(bk.reshape(2, 128).T).astype(np.float32)
    bv_s = np.ascontiguousarray(bv.reshape(2, 128).T).astype(np.float32)
    bo_s = np.ascontiguousarray(bo.reshape(2, 128).T).astype(np.float32)
    ident = np.eye(128, dtype=np.float16)

    def part(arr_cxn, npos):  # (C, rows, cols) -> (128, 2, rows*cols)
        return np.ascontiguousarray(
            arr_cxn.reshape(2, 128, npos).transpose(1, 0, 2)).astype(np.float32)

    maps = []
    for b in range(B):
        for q in range(NQ):
            r0 = RQ * q
            m = {
                "query": part(query[b, :, r0:r0 + RQ, :].reshape(C, NPOS), NPOS),
                "keypad": part(keypad_full[b, :, r0:r0 + KROWS, :].reshape(C, KFREE), KFREE),
                "peq": part(pe_q[:, r0:r0 + RQ, :].reshape(C, NPOS), NPOS).astype(np.float16),
                "pek": part(pe_k[:, r0:r0 + KROWS, :].reshape(C, KFREE), KFREE).astype(np.float16),
                "cf": part(cf_full[:, r0:r0 + RQ, :].reshape(C, NPOS), NPOS),
                "wq": wqT, "wk": wkT, "wv": wvT, "wo": woT, "wf": wfT,
                "bq": bq_s, "bk": bk_s, "bv": bv_s, "bo": bo_s,
                "ident": ident,
            }
            maps.append(m)
    return maps


def kernel(key, query, Wq, bq, Wk, bk, Wv, bv, Wo, bo, Wf, _trace=False):
    from concourse.bass_utils import run_bass_kernel_spmd

    args = [np.asarray(a, dtype=np.float32) for a in
            (key, query, Wq, bq, Wk, bk, Wv, bv, Wo, bo, Wf)]
    nc = _build_module()
    maps = _in_maps(*args)
    res = run_bass_kernel_spmd(nc, maps, list(range(8)), trace=_trace)
    _CACHE["last_res"] = res

    out = np.zeros((B, C, H, W), dtype=np.float32)
    vo = np.zeros((B, C, H, W), dtype=np.float32)
    for b in range(B):
        for q in range(NQ):
            r = res.results[b * NQ + q]
            r0 = RQ * q
            out[b, :, r0:r0 + RQ, :] = r["out_part"].transpose(1, 0, 2).reshape(C, RQ, W)
            vo[b, :, r0:r0 + RQ, :] = r["vo_part"].transpose(1, 0, 2).reshape(C, RQ, W)
    return out, vo



# revision 5
# speedup vs baseline: 1.1174x; 1.1174x over previous
"""Trainium2 Bass kernel for CrossModalMultiHeadAttentionK.

Computation (see reference): per-channel 7x7 local attention on a 40x40 grid,
B=2, C=256, with 1x1 convs (q/k/v/out/fuse) and sinusoidal positional
encodings. Sharding: 8 cores = (batch b in {0,1}) x (row-quarter q in {0..3},
10 output rows each). Each core holds all 256 channels in SBUF layout
[128 partitions, 2 channel-slots, spatial] so elementwise attention ops run
with free-dim 800 and no cross-core collectives are needed.

Version 2 changes vs the 112us baseline:
 - positional encodings folded host-side into qpe=query+pe_q, kpe=key+pe_k:
   kills peq/pek/cf DMAs, halves q/k conv matmul count, removes the fuse-conv
   correction add.
 - fp16 end-to-end: all inputs DMA'd as fp16 (no on-device casts), outputs
   DMA'd as fp16 and cast to fp32 host-side. Halves DMA bytes.
 - exp paired: one ACT instruction per TWO window offsets ([128,1600])
   amortizes the ~125ns/instr ACT overhead.
 - reciprocal_approx_fast instead of reciprocal (5x faster, 18 bits).
 - j-loop ordered even-dj pairs first so the +1-shifted k/v copies (needed
   for odd offsets' 4B alignment) are off the critical path; shifts run on
   GpSimd which is otherwise idle.
"""

import math
import numpy as np

# ---- problem constants (hardcoded per harness contract) ----
B, C, H, W = 2, 256, 40, 40
KS, PAD = 7, 3
HEAD_DIM = 32
SCALING = HEAD_DIM ** -0.5
TEMPERATURE, PESCALE, EPS = 10000.0, 2.0 * math.pi, 1e-6
NQ = 4                 # row-quarters
RQ = H // NQ           # 10 output rows per core
NPOS = RQ * W          # 400 output positions per slot
KROWS = RQ + KS - 1    # 16 padded rows needed
KW = W + 2 * PAD       # 46 padded cols
KFREE = KROWS * KW     # 736
NF = 800               # 2 slots * NPOS, elementwise free dim
NJ = KS * KS           # 49 window offsets

_CACHE = {}


def _sine_pe(mask):
    """numpy port of reference.sine_pe; mask (b,h,w) bool."""
    nm = (~mask).astype(np.float32)
    y = np.cumsum(nm, axis=1, dtype=np.float32)
    x = np.cumsum(nm, axis=2, dtype=np.float32)
    y = y / (y[:, -1:, :] + EPS) * PESCALE
    x = x / (x[:, :, -1:] + EPS) * PESCALE
    nf = C // 2
    i = np.arange(nf, dtype=np.float32)
    dim_t = (TEMPERATURE ** (2.0 * np.floor(i / 2.0) / nf)).astype(np.float32)
    px = (x[..., None] / dim_t).astype(np.float32)
    py = (y[..., None] / dim_t).astype(np.float32)

    def interleave(p):
        s = np.stack([np.sin(p[..., 0::2]), np.cos(p[..., 1::2])], axis=4)
        return s.reshape(p.shape[0], p.shape[1], p.shape[2], -1)

    pos = np.concatenate([interleave(py), interleave(px)], axis=3)
    return pos.transpose(0, 3, 1, 2).astype(np.float32)  # (b, C, h, w)


def _pe_constants():
    if "pe" in _CACHE:
        return _CACHE["pe"]
    mask_q = np.zeros((1, H, W), dtype=bool)
    pe_q = _sine_pe(mask_q)[0]  # (C, H, W)
    Hp, Wp = H + 2 * PAD, W + 2 * PAD
    mask_k = np.zeros((1, Hp, Wp), dtype=bool)
    mask_k[:, :PAD, :] = True
    mask_k[:, :, :PAD] = True
    mask_k[:, Hp - PAD:, :] = True
    mask_k[:, :, Wp - PAD:] = True
    pe_k = _sine_pe(mask_k)[0]  # (C, Hp, Wp)
    _CACHE["pe"] = (pe_q, pe_k)
    return pe_q, pe_k


# 49 offsets, even-dj pairs first (only need unshifted k_b/v_b), then odd-dj
# pairs, final odd solo. Accumulation into PSUM is order-independent.
def _j_order():
    evens = [di * KS + dj for dj in range(0, KS, 2) for di in range(KS)]
    odds = [di * KS + dj for dj in range(1, KS, 2) for di in range(KS)]
    js = evens + odds  # 28 + 21
    pairs = [(js[2 * t], js[2 * t + 1]) for t in range(24)]
    return pairs, js[48]


def _build_module():
    """Build (once) the per-core Bacc module. Same NEFF on all 8 cores."""
    if "nc" in _CACHE:
        return _CACHE["nc"]
    import concourse.bacc as bacc
    import concourse.tile as tile
    import concourse.mybir as mybir

    f32 = mybir.dt.float32
    f16 = mybir.dt.float16
    AF = mybir.ActivationFunctionType

    nc = bacc.Bacc("TRN2", target_bir_lowering=False, debug=False,
                   enable_asserts=True, num_devices=8)

    din = {}
    for name, shape, dt in [
        ("qpe", [128, 2, NPOS], f16),
        ("kpe", [128, 2, KFREE], f16),
        ("kpad", [128, 2, KFREE], f16),
        ("wq", [2, 128, 256], f16),
        ("wk", [2, 128, 256], f16),
        ("wv", [2, 128, 256], f16),
        ("wo", [2, 128, 256], f16),
        ("wf", [4, 128, 256], f16),
        ("bq", [128, 2], f32),
        ("bk", [128, 2], f32),
        ("bv", [128, 2], f32),
        ("bo", [128, 2], f32),
        ("ident", [128, 128], f16),
    ]:
        din[name] = nc.dram_tensor(name, shape, dt, kind="ExternalInput").ap()
    d_out = nc.dram_tensor("out16", [128, 2, NPOS], f16, kind="ExternalOutput").ap()
    d_vo = nc.dram_tensor("vo16", [128, 2, NPOS], f16, kind="ExternalOutput").ap()

    with tile.TileContext(nc) as tc:
        with tc.tile_pool(name="consts", bufs=1) as cp, \
             tc.tile_pool(name="work", bufs=1) as wp, \
             tc.tile_pool(name="sje", bufs=5) as sp, \
             tc.tile_pool(name="psacc", bufs=1, space="PSUM") as pa, \
             tc.tile_pool(name="psconv", bufs=2, space="PSUM") as pc:

            # ---- load inputs; priority order per ring: compute-critical
            # first, tail-only weights (wo/wf/bo) last ----
            sb = {}

            def load(eng, name, shape, dt, src=None, tag=None):
                t = cp.tile(shape, dt, tag=tag or name)
                eng.dma_start(out=t[:], in_=din[name][:] if src is None else src)
                return t

            sb["qpe"] = load(nc.sync, "qpe", [128, 2, NPOS], f16)
            sb["kpe"] = load(nc.gpsimd, "kpe", [128, 2, KFREE], f16)
            sb["kpad"] = load(nc.scalar, "kpad", [128, 2, KFREE], f16)
            for eng, name, nk in [(nc.sync, "wq", 2), (nc.gpsimd, "wk", 2),
                                  (nc.scalar, "wv", 2)]:
                sb[name] = [load(eng, name, [128, 256], f16, src=din[name][k],
                                 tag=f"{name}{k}") for k in range(nk)]
            for eng, name in [(nc.sync, "bq"), (nc.gpsimd, "bk"),
                              (nc.scalar, "bv")]:
                sb[name] = load(eng, name, [128, 2], f32)
            sb["ident"] = load(nc.sync, "ident", [128, 128], f16)
            # tail-only weights, issued last
            sb["wo"] = [load(nc.sync, "wo", [128, 256], f16, src=din["wo"][k],
                             tag=f"wo{k}") for k in range(2)]
            sb["wf"] = [load(nc.gpsimd, "wf", [128, 256], f16, src=din["wf"][k],
                             tag=f"wf{k}") for k in range(4)]
            sb["bo"] = load(nc.scalar, "bo", [128, 2], f32)

            # ---- q/k/v convs (pe pre-folded host-side) ----
            q_b = wp.tile([128, NF], f16, tag="q_b")
            k_b = wp.tile([128, 2 * KFREE], f16, tag="k_b")
            k_b1 = wp.tile([128, 2 * KFREE], f16, tag="k_b1")
            v_b = wp.tile([128, 2 * KFREE], f16, tag="v_b")
            v_b1 = wp.tile([128, 2 * KFREE], f16, tag="v_b1")

            for o in range(2):
                ps = pc.tile([128, NPOS], f32, tag="convps")
                for k in range(2):
                    nc.tensor.matmul(ps[:], sb["wq"][k][:, o * 128:(o + 1) * 128],
                                     sb["qpe"][:, k, :], start=(k == 0), stop=(k == 1))
                nc.scalar.activation(out=q_b[:, o * NPOS:(o + 1) * NPOS], in_=ps[:],
                                     func=AF.Identity, bias=sb["bq"][:, o:o + 1])

            for src, wname, bias, dest, dest1 in [
                ("kpe", "wk", "bk", k_b, k_b1),
                ("kpad", "wv", "bv", v_b, v_b1),
            ]:
                for o in range(2):
                    ps = pc.tile([128, KFREE], f32, tag="convps")
                    # psum chunks must not straddle the 2KB bank boundary
                    for sl in (slice(0, 512), slice(512, KFREE)):
                        for k in range(2):
                            nc.tensor.matmul(ps[:, sl],
                                             sb[wname][k][:, o * 128:(o + 1) * 128],
                                             sb[src][:, k, sl],
                                             start=(k == 0), stop=(k == 1))
                    nc.scalar.activation(out=dest[:, o * KFREE:(o + 1) * KFREE],
                                         in_=ps[:], func=AF.Identity,
                                         bias=sb[bias][:, o:o + 1])
                # shifted-by-one fp16 copy for odd window offsets (4B align
                # for DVE 2x mode); GpSimd is idle here and off critical path
                nc.gpsimd.tensor_copy(dest1[:, 0:2 * KFREE - 1], dest[:, 1:2 * KFREE])

            # ---- attention j-loop, paired exp ----
            num_ps = [pa.tile([128, NPOS], f32, tag=f"num{h}", name=f"num{h}")
                      for h in range(2)]
            den_ps = [pa.tile([128, NPOS], f32, tag=f"den{h}", name=f"den{h}")
                      for h in range(2)]
            q4 = q_b[:].rearrange("p (a r c) -> p a r c", a=2, r=RQ)
            k4 = k_b[:].rearrange("p (a r c) -> p a r c", a=2, r=KROWS)
            k41 = k_b1[:].rearrange("p (a r c) -> p a r c", a=2, r=KROWS)
            v4 = v_b[:].rearrange("p (a r c) -> p a r c", a=2, r=KROWS)
            v41 = v_b1[:].rearrange("p (a r c) -> p a r c", a=2, r=KROWS)

            def kwin(j, which):
                di, dj = j // KS, j % KS
                if dj % 2 == 0:
                    t4, c0 = (k4 if which == "k" else v4), dj
                else:
                    t4, c0 = (k41 if which == "k" else v41), dj - 1
                return t4[:, :, di:di + RQ, c0:c0 + W]

            pairs, solo = _j_order()
            nmm = [0]

            def acc_matmuls(p_view, e_view):
                # p_view/e_view: [128, NF] fp16 slices for one j
                for hh in range(2):
                    sl = slice(hh * NPOS, (hh + 1) * NPOS)
                    nc.tensor.matmul(num_ps[hh][:], sb["ident"][:], p_view[:, sl],
                                     start=(nmm[0] == 0), stop=(nmm[0] == NJ - 1))
                    nc.tensor.matmul(den_ps[hh][:], sb["ident"][:], e_view[:, sl],
                                     start=(nmm[0] == 0), stop=(nmm[0] == NJ - 1))
                nmm[0] += 1

            for ja, jb in pairs:
                s_t = sp.tile([128, 2 * NF], f16, tag="s")
                s5 = s_t[:].rearrange("p (x a r c) -> p x a r c", x=2, a=2, r=RQ)
                nc.vector.tensor_mul(s5[:, 0], q4, kwin(ja, "k"))
                nc.vector.tensor_mul(s5[:, 1], q4, kwin(jb, "k"))
                e_t = sp.tile([128, 2 * NF], f16, tag="e")
                nc.scalar.activation(out=e_t[:], in_=s_t[:], func=AF.Exp)
                e5 = e_t[:].rearrange("p (x a r c) -> p x a r c", x=2, a=2, r=RQ)
                p_t = sp.tile([128, 2 * NF], f16, tag="pp")
                p5 = p_t[:].rearrange("p (x a r c) -> p x a r c", x=2, a=2, r=RQ)
                nc.vector.tensor_mul(p5[:, 0], e5[:, 0], kwin(ja, "v"))
                nc.vector.tensor_mul(p5[:, 1], e5[:, 1], kwin(jb, "v"))
                acc_matmuls(p_t[:, 0:NF], e_t[:, 0:NF])
                acc_matmuls(p_t[:, NF:2 * NF], e_t[:, NF:2 * NF])

            # solo last offset (reuses pair-shaped tiles, half-filled)
            s_t = sp.tile([128, 2 * NF], f16, tag="s")
            s5 = s_t[:].rearrange("p (x a r c) -> p x a r c", x=2, a=2, r=RQ)
            nc.vector.tensor_mul(s5[:, 0], q4, kwin(solo, "k"))
            e_t = sp.tile([128, 2 * NF], f16, tag="e")
            nc.scalar.activation(out=e_t[:, 0:NF], in_=s_t[:, 0:NF], func=AF.Exp)
            e5 = e_t[:].rearrange("p (x a r c) -> p x a r c", x=2, a=2, r=RQ)
            p_t = sp.tile([128, 2 * NF], f16, tag="pp")
            p5 = p_t[:].rearrange("p (x a r c) -> p x a r c", x=2, a=2, r=RQ)
            nc.vector.tensor_mul(p5[:, 0], e5[:, 0], kwin(solo, "v"))
            acc_matmuls(p_t[:, 0:NF], e_t[:, 0:NF])

            # ---- normalize + vo conv + fuse conv, pipelined by spatial half ----
            HC = NPOS // 2  # 200-position chunks
            r_t = wp.tile([128, NF], f32, tag="r")
            att = wp.tile([128, NF], f16, tag="att")
            vo_sb = wp.tile([128, NF], f16, tag="vo")
            out_sb = wp.tile([128, NF], f16, tag="out")
            for cch in range(2):
                cs = slice(cch * HC, (cch + 1) * HC)
                for hh in range(2):
                    sl = slice(hh * NPOS + cch * HC, hh * NPOS + (cch + 1) * HC)
                    nc.vector.reciprocal_approx_fast(r_t[:, sl], den_ps[hh][:, cs])
                    nc.vector.tensor_mul(att[:, sl], num_ps[hh][:, cs], r_t[:, sl])
                for o in range(2):
                    ps = pc.tile([128, HC], f32, tag="convps", name="tailps")
                    for k in range(2):
                        nc.tensor.matmul(ps[:], sb["wo"][k][:, o * 128:(o + 1) * 128],
                                         att[:, k * NPOS + cch * HC:
                                             k * NPOS + (cch + 1) * HC],
                                         start=(k == 0), stop=(k == 1))
                    nc.scalar.activation(
                        out=vo_sb[:, o * NPOS + cch * HC:o * NPOS + (cch + 1) * HC],
                        in_=ps[:], func=AF.Identity, bias=sb["bo"][:, o:o + 1])
                for o in range(2):
                    ps = pc.tile([128, HC], f32, tag="convps", name="tailps")
                    i = 0
                    for k in range(2):
                        nc.tensor.matmul(ps[:], sb["wf"][k][:, o * 128:(o + 1) * 128],
                                         sb["qpe"][:, k, cs],
                                         start=(i == 0), stop=False)
                        i += 1
                    for k in range(2):
                        nc.tensor.matmul(ps[:], sb["wf"][2 + k][:, o * 128:(o + 1) * 128],
                                         vo_sb[:, k * NPOS + cch * HC:
                                               k * NPOS + (cch + 1) * HC],
                                         start=False, stop=(i == 3))
                        i += 1
                    nc.scalar.activation(
                        out=out_sb[:, o * NPOS + cch * HC:o * NPOS + (cch + 1) * HC],
                        in_=ps[:], func=AF.Copy)
                nc.sync.dma_start(out=d_vo[:, :, cs], in_=vo_sb[:].rearrange(
                    "p (a n) -> p a n", a=2)[:, :, cs])
                nc.sync.dma_start(out=d_out[:, :, cs], in_=out_sb[:].rearrange(
                    "p (a n) -> p a n", a=2)[:, :, cs])

    nc.compile()
    _CACHE["nc"] = nc
    return nc


def _in_maps(key, query, Wq, bq, Wk, bk, Wv, bv, Wo, bo, Wf):
    pe_q, pe_k = _pe_constants()
    kpad_full = np.pad(key, ((0, 0), (0, 0), (PAD, PAD), (PAD, PAD)))
    kpe_full = kpad_full + pe_k[None]
    qpe_full = query + pe_q[None]
    wqT = np.ascontiguousarray((Wq.T * SCALING).reshape(2, 128, 256)).astype(np.float16)
    wkT = np.ascontiguousarray(Wk.T.reshape(2, 128, 256)).astype(np.float16)
    wvT = np.ascontiguousarray(Wv.T.reshape(2, 128, 256)).astype(np.float16)
    woT = np.ascontiguousarray(Wo.T.reshape(2, 128, 256)).astype(np.float16)
    wfT = np.ascontiguousarray(Wf.T.reshape(4, 128, 256)).astype(np.float16)
    bq_s = np.ascontiguousarray((bq * SCALING).reshape(2, 128).T).astype(np.float32)
    bk_s = np.ascontiguousarray(bk.reshape(2, 128).T).astype(np.float32)
    bv_s = np.ascontiguousarray(bv.reshape(2, 128).T).astype(np.float32)
    bo_s = np.ascontiguousarray(bo.reshape(2, 128).T).astype(np.float32)
    ident = np.eye(128, dtype=np.float16)

    def part16(arr_cxn, npos):  # (C, rows*cols) -> (128, 2, rows*cols) fp16
        return np.ascontiguousarray(
            arr_cxn.reshape(2, 128, npos).transpose(1, 0, 2)).astype(np.float16)

    maps = []
    for b in range(B):
        for q in range(NQ):
            r0 = RQ * q
            m = {
                "qpe": part16(qpe_full[b, :, r0:r0 + RQ, :].reshape(C, NPOS), NPOS),
                "kpe": part16(kpe_full[b, :, r0:r0 + KROWS, :].reshape(C, KFREE), KFREE),
                "kpad": part16(kpad_full[b, :, r0:r0 + KROWS, :].reshape(C, KFREE), KFREE),
                "wq": wqT, "wk": wkT, "wv": wvT, "wo": woT, "wf": wfT,
                "bq": bq_s, "bk": bk_s, "bv": bv_s, "bo": bo_s,
                "ident": ident,
            }
            maps.append(m)
    return maps


def kernel(key, query, Wq, bq, Wk, bk, Wv, bv, Wo, bo, Wf, _trace=False):
    from concourse.bass_utils import run_bass_kernel_spmd

    args = [np.asarray(a, dtype=np.float32) for a in
            (key, query, Wq, bq, Wk, bk, Wv, bv, Wo, bo, Wf)]
    nc = _build_module()
    maps = _in_maps(*args)
    res = run_bass_kernel_spmd(nc, maps, list(range(8)), trace=_trace)
    _CACHE["last_res"] = res

    out = np.zeros((B, C, H, W), dtype=np.float32)
    vo = np.zeros((B, C, H, W), dtype=np.float32)
    for b in range(B):
        for q in range(NQ):
            r = res.results[b * NQ + q]
            r0 = RQ * q
            out[b, :, r0:r0 + RQ, :] = r["out16"].astype(np.float32).transpose(
                1, 0, 2).reshape(C, RQ, W)
            vo[b, :, r0:r0 + RQ, :] = r["vo16"].astype(np.float32).transpose(
                1, 0, 2).reshape(C, RQ, W)
    return out, vo


# revision 6
# speedup vs baseline: 1.2339x; 1.1042x over previous
"""Trainium2 Bass kernel for CrossModalMultiHeadAttentionK.

Computation (see reference): per-channel 7x7 local attention on a 40x40 grid,
B=2, C=256, with 1x1 convs (q/k/v/out/fuse) and sinusoidal positional
encodings. Sharding: 8 cores = (batch b in {0,1}) x (row-quarter q in {0..3},
10 output rows each). Each core holds all 256 channels in SBUF layout
[128 partitions, 2 channel-slots, spatial] so elementwise attention ops run
with free-dim 800 and no cross-core collectives are needed.

Version 2 changes vs the 112us baseline:
 - positional encodings folded host-side into qpe=query+pe_q, kpe=key+pe_k:
   kills peq/pek/cf DMAs, halves q/k conv matmul count, removes the fuse-conv
   correction add.
 - fp16 end-to-end: all inputs DMA'd as fp16 (no on-device casts), outputs
   DMA'd as fp16 and cast to fp32 host-side. Halves DMA bytes.
 - exp paired: one ACT instruction per TWO window offsets ([128,1600])
   amortizes the ~125ns/instr ACT overhead.
 - reciprocal_approx_fast instead of reciprocal (5x faster, 18 bits).
 - j-loop ordered even-dj pairs first so the +1-shifted k/v copies (needed
   for odd offsets' 4B alignment) are off the critical path; shifts run on
   GpSimd which is otherwise idle.
"""

import math
import numpy as np

# ---- problem constants (hardcoded per harness contract) ----
B, C, H, W = 2, 256, 40, 40
KS, PAD = 7, 3
HEAD_DIM = 32
SCALING = HEAD_DIM ** -0.5
TEMPERATURE, PESCALE, EPS = 10000.0, 2.0 * math.pi, 1e-6
NQ = 4                 # row-quarters
RQ = H // NQ           # 10 output rows per core
NPOS = RQ * W          # 400 output positions per slot
KROWS = RQ + KS - 1    # 16 padded rows needed
KW = W + 2 * PAD       # 46 padded cols
KFREE = KROWS * KW     # 736
NF = 800               # 2 slots * NPOS, elementwise free dim
NJ = KS * KS           # 49 window offsets

_CACHE = {}


def _sine_pe(mask):
    """numpy port of reference.sine_pe; mask (b,h,w) bool."""
    nm = (~mask).astype(np.float32)
    y = np.cumsum(nm, axis=1, dtype=np.float32)
    x = np.cumsum(nm, axis=2, dtype=np.float32)
    y = y / (y[:, -1:, :] + EPS) * PESCALE
    x = x / (x[:, :, -1:] + EPS) * PESCALE
    nf = C // 2
    i = np.arange(nf, dtype=np.float32)
    dim_t = (TEMPERATURE ** (2.0 * np.floor(i / 2.0) / nf)).astype(np.float32)
    px = (x[..., None] / dim_t).astype(np.float32)
    py = (y[..., None] / dim_t).astype(np.float32)

    def interleave(p):
        s = np.stack([np.sin(p[..., 0::2]), np.cos(p[..., 1::2])], axis=4)
        return s.reshape(p.shape[0], p.shape[1], p.shape[2], -1)

    pos = np.concatenate([interleave(py), interleave(px)], axis=3)
    return pos.transpose(0, 3, 1, 2).astype(np.float32)  # (b, C, h, w)


def _pe_constants():
    if "pe" in _CACHE:
        return _CACHE["pe"]
    mask_q = np.zeros((1, H, W), dtype=bool)
    pe_q = _sine_pe(mask_q)[0]  # (C, H, W)
    Hp, Wp = H + 2 * PAD, W + 2 * PAD
    mask_k = np.zeros((1, Hp, Wp), dtype=bool)
    mask_k[:, :PAD, :] = True
    mask_k[:, :, :PAD] = True
    mask_k[:, Hp - PAD:, :] = True
    mask_k[:, :, Wp - PAD:] = True
    pe_k = _sine_pe(mask_k)[0]  # (C, Hp, Wp)
    _CACHE["pe"] = (pe_q, pe_k)
    return pe_q, pe_k


# 49 offsets, even-dj pairs first (only need unshifted k_b/v_b), then odd-dj
# pairs, final odd solo. Accumulation into PSUM is order-independent.
def _j_order():
    evens = [di * KS + dj for dj in range(0, KS, 2) for di in range(KS)]
    odds = [di * KS + dj for dj in range(1, KS, 2) for di in range(KS)]
    js = evens + odds  # 28 + 21
    pairs = [(js[2 * t], js[2 * t + 1]) for t in range(24)]
    return pairs, js[48]


def _build_module():
    """Build (once) the per-core Bacc module. Same NEFF on all 8 cores."""
    if "nc" in _CACHE:
        return _CACHE["nc"]
    import concourse.bacc as bacc
    import concourse.tile as tile
    import concourse.mybir as mybir

    f32 = mybir.dt.float32
    f16 = mybir.dt.float16
    AF = mybir.ActivationFunctionType

    nc = bacc.Bacc("TRN2", target_bir_lowering=False, debug=False,
                   enable_asserts=True, num_devices=8)

    din = {}
    for name, shape, dt in [
        ("qpe", [128, 2, NPOS], f16),
        ("kpe", [128, 2, KFREE], f16),
        ("kpad", [128, 2, KFREE], f16),
        ("wblob", [128, 12, 256], f16),
        ("bblob", [128, 8], f32),
        ("ident", [128, 128], f16),
    ]:
        din[name] = nc.dram_tensor(name, shape, dt, kind="ExternalInput").ap()
    d_out = nc.dram_tensor("out16", [128, 2, NPOS], f16, kind="ExternalOutput").ap()
    d_vo = nc.dram_tensor("vo16", [128, 2, NPOS], f16, kind="ExternalOutput").ap()

    with tile.TileContext(nc) as tc:
        with tc.tile_pool(name="consts", bufs=1) as cp, \
             tc.tile_pool(name="work", bufs=1) as wp, \
             tc.tile_pool(name="sje", bufs=5) as sp, \
             tc.tile_pool(name="psacc", bufs=1, space="PSUM") as pa, \
             tc.tile_pool(name="psconv", bufs=2, space="PSUM") as pc:

            # ---- load inputs; priority order per ring: compute-critical
            # first, tail-only weights (wo/wf/bo) last ----
            sb = {}

            def load(eng, name, shape, dt, src=None, tag=None):
                t = cp.tile(shape, dt, tag=tag or name)
                eng.dma_start(out=t[:], in_=din[name][:] if src is None else src)
                return t

            sb["qpe"] = load(nc.sync, "qpe", [128, 2, NPOS], f16)
            sb["kpe"] = load(nc.gpsimd, "kpe", [128, 2, KFREE], f16)
            sb["kpad"] = load(nc.scalar, "kpad", [128, 2, KFREE], f16)
            wblob = load(nc.sync, "wblob", [128, 12, 256], f16)
            bblob = load(nc.sync, "bblob", [128, 8], f32)
            sb["ident"] = load(nc.gpsimd, "ident", [128, 128], f16)
            # weight order in blob: wq0 wq1 wk0 wk1 wv0 wv1 wo0 wo1 wf0..wf3
            for i, name in enumerate(["wq", "wk", "wv", "wo"]):
                sb[name] = [wblob[:, 2 * i + k, :] for k in range(2)]
            sb["wf"] = [wblob[:, 8 + k, :] for k in range(4)]
            for i, name in enumerate(["bq", "bk", "bv", "bo"]):
                sb[name] = bblob[:, 2 * i:2 * i + 2]

            # ---- q/k/v convs (pe pre-folded host-side) ----
            q_b = wp.tile([128, NF], f16, tag="q_b")
            k_b = wp.tile([128, 2 * KFREE], f16, tag="k_b")
            k_b1 = wp.tile([128, 2 * KFREE], f16, tag="k_b1")
            v_b = wp.tile([128, 2 * KFREE], f16, tag="v_b")
            v_b1 = wp.tile([128, 2 * KFREE], f16, tag="v_b1")

            for o in range(2):
                ps = pc.tile([128, NPOS], f32, tag="convps")
                for k in range(2):
                    nc.tensor.matmul(ps[:], sb["wq"][k][:, o * 128:(o + 1) * 128],
                                     sb["qpe"][:, k, :], start=(k == 0), stop=(k == 1))
                nc.scalar.activation(out=q_b[:, o * NPOS:(o + 1) * NPOS], in_=ps[:],
                                     func=AF.Identity, bias=sb["bq"][:, o:o + 1])

            for src, wname, bias, dest, dest1 in [
                ("kpe", "wk", "bk", k_b, k_b1),
                ("kpad", "wv", "bv", v_b, v_b1),
            ]:
                for o in range(2):
                    ps = pc.tile([128, KFREE], f32, tag="convps")
                    # psum chunks must not straddle the 2KB bank boundary
                    for sl in (slice(0, 512), slice(512, KFREE)):
                        for k in range(2):
                            nc.tensor.matmul(ps[:, sl],
                                             sb[wname][k][:, o * 128:(o + 1) * 128],
                                             sb[src][:, k, sl],
                                             start=(k == 0), stop=(k == 1))
                    nc.scalar.activation(out=dest[:, o * KFREE:(o + 1) * KFREE],
                                         in_=ps[:], func=AF.Identity,
                                         bias=sb[bias][:, o:o + 1])
                # shifted-by-one fp16 copy for odd window offsets (4B align
                # for DVE 2x mode); DVE is idle during the preamble
                nc.vector.tensor_copy(dest1[:, 0:2 * KFREE - 1], dest[:, 1:2 * KFREE])

            # ---- attention j-loop, paired exp ----
            num_ps = [pa.tile([128, NPOS], f32, tag=f"num{h}", name=f"num{h}")
                      for h in range(2)]
            den_ps = [pa.tile([128, NPOS], f32, tag=f"den{h}", name=f"den{h}")
                      for h in range(2)]
            q4 = q_b[:].rearrange("p (a r c) -> p a r c", a=2, r=RQ)
            k4 = k_b[:].rearrange("p (a r c) -> p a r c", a=2, r=KROWS)
            k41 = k_b1[:].rearrange("p (a r c) -> p a r c", a=2, r=KROWS)
            v4 = v_b[:].rearrange("p (a r c) -> p a r c", a=2, r=KROWS)
            v41 = v_b1[:].rearrange("p (a r c) -> p a r c", a=2, r=KROWS)

            def kwin(j, which):
                di, dj = j // KS, j % KS
                if dj % 2 == 0:
                    t4, c0 = (k4 if which == "k" else v4), dj
                else:
                    t4, c0 = (k41 if which == "k" else v41), dj - 1
                return t4[:, :, di:di + RQ, c0:c0 + W]

            pairs, solo = _j_order()
            nmm = [0]

            def acc_matmuls(p_view, e_view):
                # p_view/e_view: [128, NF] fp16 slices for one j
                for hh in range(2):
                    sl = slice(hh * NPOS, (hh + 1) * NPOS)
                    nc.tensor.matmul(num_ps[hh][:], sb["ident"][:], p_view[:, sl],
                                     start=(nmm[0] == 0), stop=(nmm[0] == NJ - 1))
                    nc.tensor.matmul(den_ps[hh][:], sb["ident"][:], e_view[:, sl],
                                     start=(nmm[0] == 0), stop=(nmm[0] == NJ - 1))
                nmm[0] += 1

            def half(t, x):
                return t[:, x * NF:(x + 1) * NF].rearrange(
                    "p (a r c) -> p a r c", a=2, r=RQ)

            for ja, jb in pairs:
                s_t = sp.tile([128, 2 * NF], f16, tag="s")
                nc.vector.tensor_mul(half(s_t, 0), q4, kwin(ja, "k"))
                nc.vector.tensor_mul(half(s_t, 1), q4, kwin(jb, "k"))
                e_t = sp.tile([128, 2 * NF], f16, tag="e")
                nc.scalar.activation(out=e_t[:], in_=s_t[:], func=AF.Exp)
                p_t = sp.tile([128, 2 * NF], f16, tag="pp")
                nc.vector.tensor_mul(half(p_t, 0), half(e_t, 0), kwin(ja, "v"))
                nc.vector.tensor_mul(half(p_t, 1), half(e_t, 1), kwin(jb, "v"))
                acc_matmuls(p_t[:, 0:NF], e_t[:, 0:NF])
                acc_matmuls(p_t[:, NF:2 * NF], e_t[:, NF:2 * NF])

            # solo last offset (reuses pair-shaped tiles, half-filled)
            s_t = sp.tile([128, 2 * NF], f16, tag="s")
            nc.vector.tensor_mul(half(s_t, 0), q4, kwin(solo, "k"))
            e_t = sp.tile([128, 2 * NF], f16, tag="e")
            nc.scalar.activation(out=e_t[:, 0:NF], in_=s_t[:, 0:NF], func=AF.Exp)
            p_t = sp.tile([128, 2 * NF], f16, tag="pp")
            nc.vector.tensor_mul(half(p_t, 0), half(e_t, 0), kwin(solo, "v"))
            acc_matmuls(p_t[:, 0:NF], e_t[:, 0:NF])

            # ---- normalize + vo conv + fuse conv, pipelined by spatial half ----
            HC = NPOS // 2  # 200-position chunks
            r_t = wp.tile([128, NF], f32, tag="r")
            att = wp.tile([128, NF], f16, tag="att")
            vo_sb = wp.tile([128, NF], f16, tag="vo")
            out_sb = wp.tile([128, NF], f16, tag="out")
            for cch in range(2):
                cs = slice(cch * HC, (cch + 1) * HC)
                for hh in range(2):
                    sl = slice(hh * NPOS + cch * HC, hh * NPOS + (cch + 1) * HC)
                    nc.vector.reciprocal_approx_fast(r_t[:, sl], den_ps[hh][:, cs])
                    nc.vector.tensor_mul(att[:, sl], num_ps[hh][:, cs], r_t[:, sl])
                for o in range(2):
                    ps = pc.tile([128, HC], f32, tag="convps", name="tailps")
                    for k in range(2):
                        nc.tensor.matmul(ps[:], sb["wo"][k][:, o * 128:(o + 1) * 128],
                                         att[:, k * NPOS + cch * HC:
                                             k * NPOS + (cch + 1) * HC],
                                         start=(k == 0), stop=(k == 1))
                    nc.scalar.activation(
                        out=vo_sb[:, o * NPOS + cch * HC:o * NPOS + (cch + 1) * HC],
                        in_=ps[:], func=AF.Identity, bias=sb["bo"][:, o:o + 1])
                for o in range(2):
                    ps = pc.tile([128, HC], f32, tag="convps", name="tailps")
                    i = 0
                    for k in range(2):
                        nc.tensor.matmul(ps[:], sb["wf"][k][:, o * 128:(o + 1) * 128],
                                         sb["qpe"][:, k, cs],
                                         start=(i == 0), stop=False)
                        i += 1
                    for k in range(2):
                        nc.tensor.matmul(ps[:], sb["wf"][2 + k][:, o * 128:(o + 1) * 128],
                                         vo_sb[:, k * NPOS + cch * HC:
                                               k * NPOS + (cch + 1) * HC],
                                         start=False, stop=(i == 3))
                        i += 1
                    nc.scalar.activation(
                        out=out_sb[:, o * NPOS + cch * HC:o * NPOS + (cch + 1) * HC],
                        in_=ps[:], func=AF.Copy)
                nc.sync.dma_start(out=d_vo[:, :, cs], in_=vo_sb[:].rearrange(
                    "p (a n) -> p a n", a=2)[:, :, cs])
                nc.sync.dma_start(out=d_out[:, :, cs], in_=out_sb[:].rearrange(
                    "p (a n) -> p a n", a=2)[:, :, cs])

    nc.compile()
    _CACHE["nc"] = nc
    return nc


def _in_maps(key, query, Wq, bq, Wk, bk, Wv, bv, Wo, bo, Wf):
    pe_q, pe_k = _pe_constants()
    kpad_full = np.pad(key, ((0, 0), (0, 0), (PAD, PAD), (PAD, PAD)))
    kpe_full = kpad_full + pe_k[None]
    qpe_full = query + pe_q[None]
    wqT = (Wq.T * SCALING).reshape(2, 128, 256)
    wkT = Wk.T.reshape(2, 128, 256)
    wvT = Wv.T.reshape(2, 128, 256)
    woT = Wo.T.reshape(2, 128, 256)
    wfT = Wf.T.reshape(4, 128, 256)
    # blob layout [128, 12, 256]: wq0 wq1 wk0 wk1 wv0 wv1 wo0 wo1 wf0..wf3
    wblob = np.ascontiguousarray(np.concatenate(
        [wqT, wkT, wvT, woT, wfT], axis=0).transpose(1, 0, 2)).astype(np.float16)
    bblob = np.ascontiguousarray(np.stack(
        [(bq * SCALING).reshape(2, 128), bk.reshape(2, 128),
         bv.reshape(2, 128), bo.reshape(2, 128)]).reshape(8, 128).T
        ).astype(np.float32)
    ident = np.eye(128, dtype=np.float16)

    def part16(arr_cxn, npos):  # (C, rows*cols) -> (128, 2, rows*cols) fp16
        return np.ascontiguousarray(
            arr_cxn.reshape(2, 128, npos).transpose(1, 0, 2)).astype(np.float16)

    maps = []
    for b in range(B):
        for q in range(NQ):
            r0 = RQ * q
            m = {
                "qpe": part16(qpe_full[b, :, r0:r0 + RQ, :].reshape(C, NPOS), NPOS),
                "kpe": part16(kpe_full[b, :, r0:r0 + KROWS, :].reshape(C, KFREE), KFREE),
                "kpad": part16(kpad_full[b, :, r0:r0 + KROWS, :].reshape(C, KFREE), KFREE),
                "wblob": wblob, "bblob": bblob, "ident": ident,
            }
            maps.append(m)
    return maps


def kernel(key, query, Wq, bq, Wk, bk, Wv, bv, Wo, bo, Wf, _trace=False):
    from concourse.bass_utils import run_bass_kernel_spmd

    args = [np.asarray(a, dtype=np.float32) for a in
            (key, query, Wq, bq, Wk, bk, Wv, bv, Wo, bo, Wf)]
    nc = _build_module()
    maps = _in_maps(*args)
    res = run_bass_kernel_spmd(nc, maps, list(range(8)), trace=_trace)
    _CACHE["last_res"] = res

    out = np.zeros((B, C, H, W), dtype=np.float32)
    vo = np.zeros((B, C, H, W), dtype=np.float32)
    for b in range(B):
        for q in range(NQ):
            r = res.results[b * NQ + q]
            r0 = RQ * q
            out[b, :, r0:r0 + RQ, :] = r["out16"].astype(np.float32).transpose(
                1, 0, 2).reshape(C, RQ, W)
            vo[b, :, r0:r0 + RQ, :] = r["vo16"].astype(np.float32).transpose(
                1, 0, 2).reshape(C, RQ, W)
    return out, vo


# revision 8
# speedup vs baseline: 1.2612x; 1.0221x over previous
"""Trainium2 Bass kernel for CrossModalMultiHeadAttentionK.

Computation (see reference): per-channel 7x7 local attention on a 40x40 grid,
B=2, C=256, with 1x1 convs (q/k/v/out/fuse) and sinusoidal positional
encodings. Sharding: 8 cores = (batch b in {0,1}) x (row-quarter q in {0..3},
10 output rows each). Each core holds all 256 channels in SBUF layout
[128 partitions, 2 channel-slots, spatial] so elementwise attention ops run
with free-dim 800 and no cross-core collectives are needed.

Version 2 changes vs the 112us baseline:
 - positional encodings folded host-side into qpe=query+pe_q, kpe=key+pe_k:
   kills peq/pek/cf DMAs, halves q/k conv matmul count, removes the fuse-conv
   correction add.
 - fp16 end-to-end: all inputs DMA'd as fp16 (no on-device casts), outputs
   DMA'd as fp16 and cast to fp32 host-side. Halves DMA bytes.
 - exp paired: one ACT instruction per TWO window offsets ([128,1600])
   amortizes the ~125ns/instr ACT overhead.
 - reciprocal_approx_fast instead of reciprocal (5x faster, 18 bits).
 - j-loop ordered even-dj pairs first so the +1-shifted k/v copies (needed
   for odd offsets' 4B alignment) are off the critical path; shifts run on
   GpSimd which is otherwise idle.
"""

import math
import numpy as np

# ---- problem constants (hardcoded per harness contract) ----
B, C, H, W = 2, 256, 40, 40
KS, PAD = 7, 3
HEAD_DIM = 32
SCALING = HEAD_DIM ** -0.5
TEMPERATURE, PESCALE, EPS = 10000.0, 2.0 * math.pi, 1e-6
NQ = 4                 # row-quarters
RQ = H // NQ           # 10 output rows per core
NPOS = RQ * W          # 400 output positions per slot
KROWS = RQ + KS - 1    # 16 padded rows needed
KW = W + 2 * PAD       # 46 padded cols
KFREE = KROWS * KW     # 736
NF = 800               # 2 slots * NPOS, elementwise free dim
NJ = KS * KS           # 49 window offsets

_CACHE = {}


def _sine_pe(mask):
    """numpy port of reference.sine_pe; mask (b,h,w) bool."""
    nm = (~mask).astype(np.float32)
    y = np.cumsum(nm, axis=1, dtype=np.float32)
    x = np.cumsum(nm, axis=2, dtype=np.float32)
    y = y / (y[:, -1:, :] + EPS) * PESCALE
    x = x / (x[:, :, -1:] + EPS) * PESCALE
    nf = C // 2
    i = np.arange(nf, dtype=np.float32)
    dim_t = (TEMPERATURE ** (2.0 * np.floor(i / 2.0) / nf)).astype(np.float32)
    px = (x[..., None] / dim_t).astype(np.float32)
    py = (y[..., None] / dim_t).astype(np.float32)

    def interleave(p):
        s = np.stack([np.sin(p[..., 0::2]), np.cos(p[..., 1::2])], axis=4)
        return s.reshape(p.shape[0], p.shape[1], p.shape[2], -1)

    pos = np.concatenate([interleave(py), interleave(px)], axis=3)
    return pos.transpose(0, 3, 1, 2).astype(np.float32)  # (b, C, h, w)


def _pe_constants():
    if "pe" in _CACHE:
        return _CACHE["pe"]
    mask_q = np.zeros((1, H, W), dtype=bool)
    pe_q = _sine_pe(mask_q)[0]  # (C, H, W)
    Hp, Wp = H + 2 * PAD, W + 2 * PAD
    mask_k = np.zeros((1, Hp, Wp), dtype=bool)
    mask_k[:, :PAD, :] = True
    mask_k[:, :, :PAD] = True
    mask_k[:, Hp - PAD:, :] = True
    mask_k[:, :, Wp - PAD:] = True
    pe_k = _sine_pe(mask_k)[0]  # (C, Hp, Wp)
    _CACHE["pe"] = (pe_q, pe_k)
    return pe_q, pe_k


# 49 offsets, even-dj pairs first (only need unshifted k_b/v_b), then odd-dj
# pairs, final odd solo. Accumulation into PSUM is order-independent.
def _j_order():
    evens = [di * KS + dj for dj in range(0, KS, 2) for di in range(KS)]
    odds = [di * KS + dj for dj in range(1, KS, 2) for di in range(KS)]
    js = evens + odds  # 28 + 21
    pairs = [(js[2 * t], js[2 * t + 1]) for t in range(24)]
    return pairs, js[48]


def _build_module():
    """Build (once) the per-core Bacc module. Same NEFF on all 8 cores."""
    if "nc" in _CACHE:
        return _CACHE["nc"]
    import concourse.bacc as bacc
    import concourse.tile as tile
    import concourse.mybir as mybir

    f32 = mybir.dt.float32
    f16 = mybir.dt.float16
    AF = mybir.ActivationFunctionType

    nc = bacc.Bacc("TRN2", target_bir_lowering=False, debug=False,
                   enable_asserts=True, num_devices=8)

    din = {}
    for name, shape, dt in [
        ("qpe", [128, 2, NPOS], f16),
        ("kpe", [128, 2, KFREE], f16),
        ("kpad", [128, 2, KFREE], f16),
        ("wqkv", [128, 6, 256], f16),
        ("wtail", [128, 6, 256], f16),
        ("bblob", [128, 8], f32),
        ("ident", [128, 128], f16),
    ]:
        din[name] = nc.dram_tensor(name, shape, dt, kind="ExternalInput").ap()
    # output layout [128, cch, slot, 200] so each spatial-half DMA is one
    # contiguous run per partition (1 descriptor each)
    d_out = nc.dram_tensor("out16", [128, 2, 2, NPOS // 2], f16,
                           kind="ExternalOutput").ap()
    d_vo = nc.dram_tensor("vo16", [128, 2, 2, NPOS // 2], f16,
                          kind="ExternalOutput").ap()

    with tile.TileContext(nc) as tc:
        with tc.tile_pool(name="consts", bufs=1) as cp, \
             tc.tile_pool(name="work", bufs=1) as wp, \
             tc.tile_pool(name="sje", bufs=5) as sp, \
             tc.tile_pool(name="psacc", bufs=1, space="PSUM") as pa, \
             tc.tile_pool(name="psconv", bufs=2, space="PSUM") as pc:

            # ---- load inputs; priority order per ring: compute-critical
            # first, tail-only weights (wo/wf/bo) last ----
            sb = {}

            def load(eng, name, shape, dt):
                t = cp.tile(shape, dt, tag=name)
                # flatten both sides: one contiguous run per partition -> one
                # DMA descriptor per partition instead of one per free chunk
                nflat = 1
                for s in shape[1:]:
                    nflat *= s
                eng.dma_start(out=t[:].rearrange("p ... -> p (...)")
                              if len(shape) > 2 else t[:],
                              in_=din[name][:].rearrange("p ... -> p (...)")
                              if len(shape) > 2 else din[name][:])
                return t

            sb["qpe"] = load(nc.sync, "qpe", [128, 2, NPOS], f16)
            wqkv = load(nc.gpsimd, "wqkv", [128, 6, 256], f16)
            sb["kpe"] = load(nc.scalar, "kpe", [128, 2, KFREE], f16)
            bblob = load(nc.sync, "bblob", [128, 8], f32)
            sb["kpad"] = load(nc.gpsimd, "kpad", [128, 2, KFREE], f16)
            sb["ident"] = load(nc.sync, "ident", [128, 128], f16)
            wtail = load(nc.scalar, "wtail", [128, 6, 256], f16)
            # wqkv order: wq0 wq1 wk0 wk1 wv0 wv1; wtail: wo0 wo1 wf0..wf3
            for i, name in enumerate(["wq", "wk", "wv"]):
                sb[name] = [wqkv[:, 2 * i + k, :] for k in range(2)]
            sb["wo"] = [wtail[:, k, :] for k in range(2)]
            sb["wf"] = [wtail[:, 2 + k, :] for k in range(4)]
            for i, name in enumerate(["bq", "bk", "bv", "bo"]):
                sb[name] = bblob[:, 2 * i:2 * i + 2]

            # ---- q/k/v convs (pe pre-folded host-side) ----
            q_b = wp.tile([128, NF], f16, tag="q_b")
            k_b = wp.tile([128, 2 * KFREE], f16, tag="k_b")
            k_b1 = wp.tile([128, 2 * KFREE], f16, tag="k_b1")
            v_b = wp.tile([128, 2 * KFREE], f16, tag="v_b")
            v_b1 = wp.tile([128, 2 * KFREE], f16, tag="v_b1")

            for o in range(2):
                ps = pc.tile([128, NPOS], f32, tag="convps")
                for k in range(2):
                    nc.tensor.matmul(ps[:], sb["wq"][k][:, o * 128:(o + 1) * 128],
                                     sb["qpe"][:, k, :], start=(k == 0), stop=(k == 1))
                nc.scalar.activation(out=q_b[:, o * NPOS:(o + 1) * NPOS], in_=ps[:],
                                     func=AF.Identity, bias=sb["bq"][:, o:o + 1])

            for src, wname, bias, dest, dest1 in [
                ("kpe", "wk", "bk", k_b, k_b1),
                ("kpad", "wv", "bv", v_b, v_b1),
            ]:
                for o in range(2):
                    ps = pc.tile([128, KFREE], f32, tag="convps")
                    # psum chunks must not straddle the 2KB bank boundary
                    for sl in (slice(0, 512), slice(512, KFREE)):
                        for k in range(2):
                            nc.tensor.matmul(ps[:, sl],
                                             sb[wname][k][:, o * 128:(o + 1) * 128],
                                             sb[src][:, k, sl],
                                             start=(k == 0), stop=(k == 1))
                    nc.scalar.activation(out=dest[:, o * KFREE:(o + 1) * KFREE],
                                         in_=ps[:], func=AF.Identity,
                                         bias=sb[bias][:, o:o + 1])
                # shifted-by-one fp16 copy for odd window offsets (4B align
                # for DVE 2x mode); DVE is idle during the preamble
                nc.vector.tensor_copy(dest1[:, 0:2 * KFREE - 1], dest[:, 1:2 * KFREE])

            # ---- attention j-loop, paired exp ----
            num_ps = [pa.tile([128, NPOS], f32, tag=f"num{h}", name=f"num{h}")
                      for h in range(2)]
            den_ps = [pa.tile([128, NPOS], f32, tag=f"den{h}", name=f"den{h}")
                      for h in range(2)]
            q4 = q_b[:].rearrange("p (a r c) -> p a r c", a=2, r=RQ)
            k4 = k_b[:].rearrange("p (a r c) -> p a r c", a=2, r=KROWS)
            k41 = k_b1[:].rearrange("p (a r c) -> p a r c", a=2, r=KROWS)
            v4 = v_b[:].rearrange("p (a r c) -> p a r c", a=2, r=KROWS)
            v41 = v_b1[:].rearrange("p (a r c) -> p a r c", a=2, r=KROWS)

            def kwin(j, which):
                di, dj = j // KS, j % KS
                if dj % 2 == 0:
                    t4, c0 = (k4 if which == "k" else v4), dj
                else:
                    t4, c0 = (k41 if which == "k" else v41), dj - 1
                return t4[:, :, di:di + RQ, c0:c0 + W]

            pairs, solo = _j_order()
            nmm = [0]

            def acc_matmuls(p_view, e_view):
                # p_view/e_view: [128, NF] fp16 slices for one j
                for hh in range(2):
                    sl = slice(hh * NPOS, (hh + 1) * NPOS)
                    nc.tensor.matmul(num_ps[hh][:], sb["ident"][:], p_view[:, sl],
                                     start=(nmm[0] == 0), stop=(nmm[0] == NJ - 1))
                    nc.tensor.matmul(den_ps[hh][:], sb["ident"][:], e_view[:, sl],
                                     start=(nmm[0] == 0), stop=(nmm[0] == NJ - 1))
                nmm[0] += 1

            def half(t, x):
                return t[:, x * NF:(x + 1) * NF].rearrange(
                    "p (a r c) -> p a r c", a=2, r=RQ)

            for ja, jb in pairs:
                s_t = sp.tile([128, 2 * NF], f16, tag="s")
                nc.vector.tensor_mul(half(s_t, 0), q4, kwin(ja, "k"))
                nc.vector.tensor_mul(half(s_t, 1), q4, kwin(jb, "k"))
                e_t = sp.tile([128, 2 * NF], f16, tag="e")
                nc.scalar.activation(out=e_t[:], in_=s_t[:], func=AF.Exp)
                p_t = sp.tile([128, 2 * NF], f16, tag="pp")
                nc.vector.tensor_mul(half(p_t, 0), half(e_t, 0), kwin(ja, "v"))
                nc.vector.tensor_mul(half(p_t, 1), half(e_t, 1), kwin(jb, "v"))
                acc_matmuls(p_t[:, 0:NF], e_t[:, 0:NF])
                acc_matmuls(p_t[:, NF:2 * NF], e_t[:, NF:2 * NF])

            # solo last offset (reuses pair-shaped tiles, half-filled)
            s_t = sp.tile([128, 2 * NF], f16, tag="s")
            nc.vector.tensor_mul(half(s_t, 0), q4, kwin(solo, "k"))
            e_t = sp.tile([128, 2 * NF], f16, tag="e")
            nc.scalar.activation(out=e_t[:, 0:NF], in_=s_t[:, 0:NF], func=AF.Exp)
            p_t = sp.tile([128, 2 * NF], f16, tag="pp")
            nc.vector.tensor_mul(half(p_t, 0), half(e_t, 0), kwin(solo, "v"))
            acc_matmuls(p_t[:, 0:NF], e_t[:, 0:NF])

            # ---- normalize + vo conv + fuse conv, pipelined by spatial half ----
            HC = NPOS // 2  # 200-position chunks
            r_t = wp.tile([128, NF], f32, tag="r")
            att = wp.tile([128, NF], f16, tag="att")
            vo_sb = wp.tile([128, NF], f16, tag="vo")
            vo_x = wp.tile([128, 2, 2, HC], f16, tag="vo_x")
            out_x = wp.tile([128, 2, 2, HC], f16, tag="out_x")
            for cch in range(2):
                cs = slice(cch * HC, (cch + 1) * HC)
                for hh in range(2):
                    sl = slice(hh * NPOS + cch * HC, hh * NPOS + (cch + 1) * HC)
                    nc.vector.reciprocal_approx_fast(r_t[:, sl], den_ps[hh][:, cs])
                    nc.vector.tensor_mul(att[:, sl], num_ps[hh][:, cs], r_t[:, sl])
                for o in range(2):
                    ps = pc.tile([128, HC], f32, tag="convps", name="tailps")
                    for k in range(2):
                        nc.tensor.matmul(ps[:], sb["wo"][k][:, o * 128:(o + 1) * 128],
                                         att[:, k * NPOS + cch * HC:
                                             k * NPOS + (cch + 1) * HC],
                                         start=(k == 0), stop=(k == 1))
                    nc.scalar.activation(
                        out=vo_sb[:, o * NPOS + cch * HC:o * NPOS + (cch + 1) * HC],
                        in_=ps[:], func=AF.Identity, bias=sb["bo"][:, o:o + 1])
                    nc.vector.tensor_copy(vo_x[:, cch, o, :],
                                          vo_sb[:, o * NPOS + cch * HC:
                                                o * NPOS + (cch + 1) * HC])
                for o in range(2):
                    ps = pc.tile([128, HC], f32, tag="convps", name="tailps")
                    i = 0
                    for k in range(2):
                        nc.tensor.matmul(ps[:], sb["wf"][k][:, o * 128:(o + 1) * 128],
                                         sb["qpe"][:, k, cs],
                                         start=(i == 0), stop=False)
                        i += 1
                    for k in range(2):
                        nc.tensor.matmul(ps[:], sb["wf"][2 + k][:, o * 128:(o + 1) * 128],
                                         vo_sb[:, k * NPOS + cch * HC:
                                               k * NPOS + (cch + 1) * HC],
                                         start=False, stop=(i == 3))
                        i += 1
                    nc.scalar.activation(
                        out=out_x[:, cch, o, :], in_=ps[:], func=AF.Copy)
                nc.sync.dma_start(out=d_vo[:, cch].rearrange("p a n -> p (a n)"),
                                  in_=vo_x[:, cch].rearrange("p a n -> p (a n)"))
                nc.sync.dma_start(out=d_out[:, cch].rearrange("p a n -> p (a n)"),
                                  in_=out_x[:, cch].rearrange("p a n -> p (a n)"))

    nc.compile()
    _CACHE["nc"] = nc
    return nc


def _in_maps(key, query, Wq, bq, Wk, bk, Wv, bv, Wo, bo, Wf):
    pe_q, pe_k = _pe_constants()
    kpad_full = np.pad(key, ((0, 0), (0, 0), (PAD, PAD), (PAD, PAD)))
    kpe_full = kpad_full + pe_k[None]
    qpe_full = query + pe_q[None]
    wqT = (Wq.T * SCALING).reshape(2, 128, 256)
    wkT = Wk.T.reshape(2, 128, 256)
    wvT = Wv.T.reshape(2, 128, 256)
    woT = Wo.T.reshape(2, 128, 256)
    wfT = Wf.T.reshape(4, 128, 256)
    wqkv = np.ascontiguousarray(np.concatenate(
        [wqT, wkT, wvT], axis=0).transpose(1, 0, 2)).astype(np.float16)
    wtail = np.ascontiguousarray(np.concatenate(
        [woT, wfT], axis=0).transpose(1, 0, 2)).astype(np.float16)
    bblob = np.ascontiguousarray(np.stack(
        [(bq * SCALING).reshape(2, 128), bk.reshape(2, 128),
         bv.reshape(2, 128), bo.reshape(2, 128)]).reshape(8, 128).T
        ).astype(np.float32)
    ident = np.eye(128, dtype=np.float16)

    def part16(arr_cxn, npos):  # (C, rows*cols) -> (128, 2, rows*cols) fp16
        return np.ascontiguousarray(
            arr_cxn.reshape(2, 128, npos).transpose(1, 0, 2)).astype(np.float16)

    maps = []
    for b in range(B):
        for q in range(NQ):
            r0 = RQ * q
            m = {
                "qpe": part16(qpe_full[b, :, r0:r0 + RQ, :].reshape(C, NPOS), NPOS),
                "kpe": part16(kpe_full[b, :, r0:r0 + KROWS, :].reshape(C, KFREE), KFREE),
                "kpad": part16(kpad_full[b, :, r0:r0 + KROWS, :].reshape(C, KFREE), KFREE),
                "wqkv": wqkv, "wtail": wtail, "bblob": bblob, "ident": ident,
            }
            maps.append(m)
    return maps


def kernel(key, query, Wq, bq, Wk, bk, Wv, bv, Wo, bo, Wf, _trace=False):
    from concourse.bass_utils import run_bass_kernel_spmd

    args = [np.asarray(a, dtype=np.float32) for a in
            (key, query, Wq, bq, Wk, bk, Wv, bv, Wo, bo, Wf)]
    nc = _build_module()
    maps = _in_maps(*args)
    res = run_bass_kernel_spmd(nc, maps, list(range(8)), trace=_trace)
    _CACHE["last_res"] = res

    out = np.zeros((B, C, H, W), dtype=np.float32)
    vo = np.zeros((B, C, H, W), dtype=np.float32)
    for b in range(B):
        for q in range(NQ):
            r = res.results[b * NQ + q]
            r0 = RQ * q

            def unpack(a):  # [128, cch2, slot2, 200] -> (C, RQ, W)
                a = np.asarray(a).astype(np.float32)
                full = np.concatenate([a[:, 0], a[:, 1]], axis=-1)  # [128,2,400]
                return full.transpose(1, 0, 2).reshape(C, RQ, W)

            out[b, :, r0:r0 + RQ, :] = unpack(r["out16"])
            vo[b, :, r0:r0 + RQ, :] = unpack(r["vo16"])
    return out, vo
